# revision 40
# baseline (speedup 1.0000x reference)
"""YOLO-style loss kernel for Trainium2, SPMD over 8 NeuronCores.

Inputs (full): pred_tensor [32768,7,7,30] f32, target_tensor [32768,7,7,30] f32.
Output: np.ndarray shape (5,) f32 = (loss_xy, loss_wh, loss_obj, loss_noobj, loss_class).

Pure data parallel on batch: each core gets 4096 samples (200704 cells),
fp16 on-chip, box-major contiguous channel groups, 4 chunks of 392
cells/partition, raw + temp pools double-buffered.

All five sum-of-squares reductions run on the otherwise-idle TensorEngine
as gram-diagonal accumulations: PSUM[j,f] += sum_p a[p,f]*b[p,j], with the
payload on (section-)diagonals; the host extracts diagonals in fp64.
 - loss_class needs no mask op at all: gram(obj_stationary, d^2_moving)
   applies the 0/1 mask inside the PE; d^2 comes from an in-place ACT
   Square; full 128-col blocks pack 4 channel rows per matmul (out
   [128,512] = one PSUM bank), the twenty 8-wide row tails are one packed
   matmul into a dedicated [128,160] region.
 - loss_xy likewise: gram(resp_row, dxy^2); single-row grams land on the
   true diagonal.
 - loss_wh/obj keep a cheap DVE premask (resp in {0,1}) + flat self-grams.
 - loss_noobj = sum(pc^2) - sum(tc*pc^2) (tc in {0,1}): two grams, one ACT
   square, zero DVE ops.
Engine placement is latency-aware: GpSimd only gets off-critical-path
input-only products (areas) - its TT throughput is ~5x below DVE and
GP<->DVE SBUF port contention taxes the DVE, so anything on the IoU chain
stays on DVE/ACT. The corner scale rides a 4x DVE tensor_scalar. Chunk 0's
class DMA is split so the first sub starts as soon as half the data lands.
"""

import os
import sys

sys.path.insert(0, "/opt/trn_rl_repo")

import numpy as np

import concourse.bass as bass
import concourse.bacc as bacc
import concourse.tile as tile
from concourse import mybir
from concourse import bass_utils

F32 = mybir.dt.float32
F16 = mybir.dt.float16
I16 = mybir.dt.int16
I32 = mybir.dt.int32
ALU = mybir.AluOpType
ACT = mybir.ActivationFunctionType

S = 7
B = 2
C = 20
D = 30
N_FULL = 32768
N_CORES = 8
N_SHARD = N_FULL // N_CORES            # 4096 samples per core
R = N_SHARD * S * S                    # 200704 cells per core
P = 128                                # partitions
RP = R // P                            # 1568 cells per partition
NCK = 392                              # cells per partition per chunk
N_CH = RP // NCK                       # 4 chunks

PERM_XY = [0, 1, 5, 6]   # x0,y0,x1,y1
PERM_WH = [2, 3, 7, 8]   # w0,h0,w1,h1
PERM_CF = [4, 9]         # c0,c1

# gram regions in PSUM: five [128,128] + class [128,512] + class-tail [128,160]
REG_XY, REG_WH, REG_OBJ, REG_PCF, REG_TCPC, REG_CLS, REG_CLT = range(7)
REG_W = [P, P, P, P, P, 4 * P, 160]      # free width per region
N_REG = 7
OUT_W = sum(REG_W)                       # 1184 fp32 columns


def _mk(ap, dims):
    """Rebuild the free dims of `ap` (keeping partition dim + offset) as
    `dims` = list of (step, count)."""
    new = [list(ap.ap[0])] + [[s, c] for s, c in dims]
    return bass.AP(tensor=ap.tensor, offset=ap.offset, ap=new)


def _ins(ap, pos, step, count):
    new = [list(x) for x in ap.ap]
    new.insert(pos, [step, count])
    return bass.AP(tensor=ap.tensor, offset=ap.offset, ap=new)


def build_program():
    nc = bacc.Bacc("TRN2", target_bir_lowering=False, debug=False)
    n = NCK

    def din(name, per_chunk):
        return nc.dram_tensor(name, [P, N_CH * per_chunk], F16, kind="ExternalInput")

    pbox, tbox = din("pbox", n * 10), din("tbox", n * 10)
    pcl, tcl = din("pcl", C * n), din("tcl", C * n)
    out = nc.dram_tensor("out", [P, OUT_W], F32, kind="ExternalOutput")

    pbox_v = pbox.ap().rearrange("p (k a) -> p k a", k=N_CH, a=n * 10)
    tbox_v = tbox.ap().rearrange("p (k a) -> p k a", k=N_CH, a=n * 10)
    pcl_v = pcl.ap().rearrange("p (k c i) -> p k c i", k=N_CH, c=C, i=n)
    tcl_v = tcl.ap().rearrange("p (k c i) -> p k c i", k=N_CH, c=C, i=n)

    # block boundaries within one n-row (392 = 128+128+128+8)
    row_blocks = []
    o = 0
    while o < n:
        w = min(128, n - o)
        row_blocks.append((o, w))
        o += w

    with tile.TileContext(nc) as tc:
        with (
            tc.tile_pool(name="raw", bufs=2) as raw,
            tc.tile_pool(name="tmp", bufs=2) as tmp,
            tc.tile_pool(name="tmg", bufs=2) as tmg,
            tc.tile_pool(name="persist", bufs=1) as persist,
            tc.tile_pool(name="ps", bufs=1, space="PSUM") as psum,
        ):
            # gram accumulation regions
            G = [psum.tile([P, REG_W[r]], F32, name=f"G{r}", tag=f"G{r}")
                 for r in range(N_REG)]
            started = [False] * N_REG

            def gram(reg, lhsT, rhs, last):
                """PSUM[reg] += lhsT.T @ rhs (diag carries the payload)."""
                m = lhsT.free_size()
                f = rhs.free_size()
                nc.tensor.matmul(
                    G[reg][0:m, 0:f], lhsT, rhs,
                    start=not started[reg], stop=last,
                )
                started[reg] = True

            # prime the ACT table (Copy/Relu/Square/Sqrt all live in the
            # sqrt-anchored set) before any real work
            warm = persist.tile([P, 1], F16)
            nc.gpsimd.memset(warm, 1.0)
            nc.scalar.activation(warm, warm, ACT.Sqrt)

            for k in range(N_CH):
                last = k == N_CH - 1
                Bp = raw.tile([P, 10 * n], F16, tag="Bp")
                Bt = raw.tile([P, 10 * n], F16, tag="Bt")
                Pcl = raw.tile([P, C, n], F16, tag="Pcl")
                Tcl = raw.tile([P, C, n], F16, tag="Tcl")
                if k == 0:
                    # xy+wh rows first: the IoU head starts ~1us earlier
                    nc.sync.dma_start(out=Bp[:, 0:8 * n], in_=pbox_v[:, k, 0:8 * n])
                    nc.sync.dma_start(out=Bt[:, 0:8 * n], in_=tbox_v[:, k, 0:8 * n])
                    nc.sync.dma_start(out=Bp[:, 8 * n:10 * n],
                                      in_=pbox_v[:, k, 8 * n:10 * n])
                    nc.sync.dma_start(out=Bt[:, 8 * n:10 * n],
                                      in_=tbox_v[:, k, 8 * n:10 * n])
                else:
                    nc.sync.dma_start(out=Bp, in_=pbox_v[:, k])
                    nc.sync.dma_start(out=Bt, in_=tbox_v[:, k])
                if k == 0:
                    for q in range(2):
                        cq = slice(10 * q, 10 * (q + 1))
                        nc.sync.dma_start(out=Pcl[:, cq, :], in_=pcl_v[:, k, cq])
                        nc.sync.dma_start(out=Tcl[:, cq, :], in_=tcl_v[:, k, cq])
                else:
                    nc.sync.dma_start(out=Pcl, in_=pcl_v[:, k])
                    nc.sync.dma_start(out=Tcl, in_=tcl_v[:, k])

                # contiguous channel-group rows
                Pxy = Bp[:, 0:4 * n]
                Pwh = Bp[:, 4 * n:8 * n]
                Pcf = Bp[:, 8 * n:10 * n]
                Txy = Bt[:, 0:4 * n]
                Twh = Bt[:, 4 * n:8 * n]
                obj_src = Bt[:, 8 * n:9 * n]    # target c0 row, [P,n]

                # ---- input-only ops first: fill every engine early ----
                t1 = tmp.tile([P, 4, n], F16, tag="t1")
                nc.vector.tensor_scalar_mul(t1, Pwh, 3.5)
                t2 = tmp.tile([P, 2, n], F16, tag="t2")
                nc.scalar.activation(t2, Bt[:, 4 * n:6 * n], ACT.Copy, scale=3.5)
                sq8 = tmp.tile([P, 8, n], F16, tag="sq8")
                nc.scalar.activation(sq8[:, 0:4, :], Pwh, ACT.Sqrt)
                nc.scalar.activation(sq8[:, 4:8, :], Twh, ACT.Sqrt)
                sqp2 = tmg.tile([P, 2, n], F16, tag="sqp2")
                nc.scalar.activation(sqp2, Pcf, ACT.Square)
                # areas on GpSimd (off the critical chain)
                areap2 = tmp.tile([P, 2, n], F16, tag="areap2")
                areat = tmp.tile([P, n], F16, tag="areat")
                pw2 = _mk(Bp[:, 4 * n], [(2 * n, 2), (1, n)])
                ph2 = _mk(Bp[:, 5 * n], [(2 * n, 2), (1, n)])
                nc.gpsimd.tensor_tensor(areap2, pw2, ph2, op=ALU.mult)
                nc.gpsimd.tensor_tensor(
                    areat, Bt[:, 4 * n:5 * n], Bt[:, 5 * n:6 * n], op=ALU.mult
                )
                def noobj_grams():
                    nb2l = (2 * n + 127) // 128
                    for bi in range(nb2l):
                        o = bi * 128
                        w = min(128, 2 * n - o)
                        gram(REG_PCF, Pcf[:, o:o + w], Pcf[:, o:o + w],
                             last and bi == nb2l - 1)
                    for r2 in range(2):
                        for bi, (o, w) in enumerate(row_blocks):
                            tcrow = Bt[:, (8 + r2) * n + o:(8 + r2) * n + o + w]
                            gram(REG_TCPC, tcrow, sqp2[:, r2, o:o + w],
                                 last and r2 == 1 and bi == len(row_blocks) - 1)

                d8 = tmg.tile([P, 8, n], F16, tag="d8")
                nc.vector.tensor_tensor(d8[:, 0:4, :], Txy, Pxy, op=ALU.subtract)
                nc.vector.tensor_tensor(
                    d8[:, 4:8, :], sq8[:, 4:8, :], sq8[:, 0:4, :], op=ALU.subtract
                )

                # ---- IoU chain (DVE-resident) ----
                nl4 = tmp.tile([P, 4, n], F16, tag="nl4")    # -(7l) both boxes
                r4 = tmp.tile([P, 4, n], F16, tag="r4")      # 7r both boxes
                nc.vector.tensor_tensor(nl4, t1, Pxy, op=ALU.subtract)
                nc.vector.tensor_tensor(r4, t1, Pxy, op=ALU.add)
                txy0 = Bt[:, 0:2 * n]
                nlt2 = tmp.tile([P, 2, n], F16, tag="nlt2")
                rt2 = tmp.tile([P, 2, n], F16, tag="rt2")
                nc.vector.tensor_tensor(nlt2, t2, txy0, op=ALU.subtract)
                nc.vector.tensor_tensor(rt2, t2, txy0, op=ALU.add)
                nlt2b = _mk(nlt2[:, 0, 0], [(0, 2), (n, 2), (1, n)])
                rt2b = _mk(rt2[:, 0, 0], [(0, 2), (n, 2), (1, n)])

                mln4 = tmp.tile([P, 4, n], F16, tag="mln4")
                mr4 = tmp.tile([P, 4, n], F16, tag="mr4")
                nc.vector.tensor_tensor(mln4, nl4, nlt2b, op=ALU.min)
                nc.vector.tensor_tensor(mr4, r4, rt2b, op=ALU.min)
                s4 = tmp.tile([P, 4, n], F16, tag="s4")
                nc.vector.tensor_tensor(s4, mln4, mr4, op=ALU.add)
                cw4 = tmp.tile([P, 4, n], F16, tag="cw4")
                nc.scalar.activation(cw4, s4, ACT.Relu, scale=1.0 / 7.0)

                # ---- class block (mid-chunk): two half-pipelines so the
                #      DVE sub of half 2 overlaps the ACT square of half 1;
                #      d^2 lands in place over Tcl ----
                H = 5 if k == 0 else C // 2
                for h in range(C // H):
                    cs = h * H
                    nc.vector.tensor_tensor(
                        Tcl[:, cs:cs + H, :], Tcl[:, cs:cs + H, :],
                        Pcl[:, cs:cs + H, :], op=ALU.subtract
                    )
                    nc.scalar.activation(
                        Tcl[:, cs:cs + H, :], Tcl[:, cs:cs + H, :], ACT.Square
                    )
                for o, w in row_blocks[:-1]:
                    for c0 in range(0, C, 4):
                        mv = _mk(Tcl[:, c0, o], [(n, 4), (1, w)])
                        gram(REG_CLS, obj_src[:, o:o + w], mv,
                             last and o == row_blocks[-2][0] and c0 == C - 4)
                to, tw = row_blocks[-1]
                mvt = _mk(Tcl[:, 0, to], [(n, C), (1, tw)])
                gram(REG_CLT, obj_src[:, to:to + tw], mvt, last)

                # ---- IoU tail ----
                inter2 = tmp.tile([P, 2, n], F16, tag="inter2")
                cwx = cw4[:, 0:4:2, :]
                cwy = cw4[:, 1:4:2, :]
                nc.vector.tensor_tensor(inter2, cwx, cwy, op=ALU.mult)
                u2h = tmp.tile([P, 2, n], F16, tag="u2h")
                u2 = tmp.tile([P, 2, n], F16, tag="u2")
                nc.vector.tensor_tensor(u2h, areap2, inter2, op=ALU.subtract)
                areatb = _ins(areat[:, :], 1, 0, 2)
                nc.vector.tensor_tensor(u2, u2h, areatb, op=ALU.add)

                from concourse.dve_ops import (
                    RECIP_APPROX_FAST_CONSTS as _RC,
                    RECIPROCAL_APPROX_FAST as _RF,
                )
                rcp16 = tmp.tile([P, 2, n], F16, tag="rcp16")
                nc.vector._custom_dve(
                    _RF, out=rcp16, in0=u2,
                    s0=_RC["s0"], s1=_RC["s1"], imm2=_RC["imm2"],
                )
                iou2 = tmp.tile([P, 2, n], F16, tag="iou2")
                nc.vector.tensor_tensor(iou2, inter2, rcp16, op=ALU.mult)

                is1 = tmp.tile([P, n], F16, tag="is1")
                riou = tmp.tile([P, n], F16, tag="riou")
                nc.vector.tensor_tensor(is1, iou2[:, 1, :], iou2[:, 0, :], op=ALU.is_gt)
                nc.vector.tensor_tensor(riou, iou2[:, 1, :], iou2[:, 0, :], op=ALU.max)

                resp = tmp.tile([P, 2, n], F16, tag="resp")
                nc.vector.tensor_tensor(resp[:, 1, :], obj_src, is1, op=ALU.mult)
                nc.vector.tensor_tensor(resp[:, 0, :], obj_src, resp[:, 1, :], op=ALU.subtract)

                # ---- xy: square d8 rows 0:4 in place (ACT), resp applied
                #      by the PE (single-row grams land on the true diag) ----
                nc.scalar.activation(d8[:, 0:4, :], d8[:, 0:4, :], ACT.Square)
                for r in range(4):
                    bx = r // 2
                    for bi, (o, w) in enumerate(row_blocks):
                        gram(REG_XY, resp[:, bx, o:o + w], d8[:, r, o:o + w],
                             last and r == 3 and bi == len(row_blocks) - 1)
                # ---- wh: premask in place + flat self-gram ----
                resp4b = _mk(resp[:, 0, 0], [(n, 2), (0, 2), (1, n)])
                nc.vector.tensor_tensor(d8[:, 4:8, :], d8[:, 4:8, :], resp4b, op=ALU.mult)
                whf = _mk(d8[:, 4, 0], [(1, 4 * n)])
                nb = (4 * n + 127) // 128
                for bi in range(nb):
                    o = bi * 128
                    w = min(128, 4 * n - o)
                    gram(REG_WH, whf[:, o:o + w], whf[:, o:o + w],
                         last and bi == nb - 1)

                # ---- obj conf: dc2 = riou - pc (GP), premask, self-gram ----
                dc2 = tmg.tile([P, 2, n], F16, tag="dc2")
                rioub = _ins(riou[:, :], 1, 0, 2)
                nc.vector.tensor_tensor(dc2, rioub, Pcf, op=ALU.subtract)
                nc.vector.tensor_tensor(dc2, dc2, resp, op=ALU.mult)
                dmc2f = _mk(dc2[:, 0, 0], [(1, 2 * n)])
                nb2 = (2 * n + 127) // 128
                for bi in range(nb2):
                    o = bi * 128
                    w = min(128, 2 * n - o)
                    gram(REG_OBJ, dmc2f[:, o:o + w], dmc2f[:, o:o + w],
                         last and bi == nb2 - 1)

                # ---- noobj: sum(pc^2) - sum(tc*pc^2), tc in {0,1} ----
                noobj_grams()

            # ---- extract: copy PSUM regions to SBUF, DMA each out as
            #      soon as its copy lands (only REG_OBJ gates the tail) ----
            og = persist.tile([P, OUT_W], F32)
            outv = out.ap()
            off = 0
            for r in range(N_REG):
                nc.scalar.activation(og[:, off:off + REG_W[r]], G[r], ACT.Copy)
                nc.sync.dma_start(out=outv[:, off:off + REG_W[r]],
                                  in_=og[:, off:off + REG_W[r]])
                off += REG_W[r]

    nc.compile()
    return nc


_nc_cache = None
LAST_EXEC_NS = None
LAST_RESULT = None


def _get_nc():
    global _nc_cache
    if _nc_cache is None:
        _nc_cache = build_program()
    return _nc_cache


def _prep(full):
    """[N*S*S, 30] f32 -> per-core fp16 (box blocks [k][xy4|wh4|cf2], cls)."""
    A = np.asarray(full, dtype=np.float32).reshape(N_CORES, P, N_CH, NCK, D)
    A16 = A.astype(np.float16)
    xy = A16[..., PERM_XY].transpose(0, 1, 2, 4, 3)
    wh = A16[..., PERM_WH].transpose(0, 1, 2, 4, 3)
    cf = A16[..., PERM_CF].transpose(0, 1, 2, 4, 3)
    box = np.ascontiguousarray(np.concatenate([xy, wh, cf], axis=-2)).reshape(
        N_CORES, P, -1
    )
    cl = np.ascontiguousarray(A16[..., 10:30].transpose(0, 1, 2, 4, 3)).reshape(
        N_CORES, P, -1
    )
    return box, cl


def kernel(pred_tensor, target_tensor):
    global LAST_EXEC_NS, LAST_RESULT
    pred = np.asarray(pred_tensor).reshape(N_FULL * S * S, D)
    tgt = np.asarray(target_tensor).reshape(N_FULL * S * S, D)

    pb, pc = _prep(pred)
    tb, tc = _prep(tgt)

    in_maps = []
    for i in range(N_CORES):
        in_maps.append({"pbox": pb[i], "tbox": tb[i], "pcl": pc[i], "tcl": tc[i]})

    nc = _get_nc()
    trace = bool(os.environ.get("KERNEL_TRACE"))
    tmpdir = os.environ.get("KERNEL_TRACE_DIR") or None
    res = bass_utils.run_bass_kernel_spmd(
        nc, in_maps, core_ids=list(range(N_CORES)), trace=trace, tmpdir=tmpdir
    )
    LAST_RESULT = res
    if res.exec_time_ns is not None:
        LAST_EXEC_NS = res.exec_time_ns
    # gram regions; (section-)diagonals carry the sums
    offs = np.cumsum([0] + REG_W)
    total = np.zeros(N_REG, dtype=np.float64)
    for m in res.results:
        o = m["out"].astype(np.float64)
        for r in range(N_REG):
            reg = o[:, offs[r]:offs[r + 1]]
            if r == REG_CLT:
                for c in range(C):
                    total[r] += np.trace(reg[:8, c * 8:(c + 1) * 8])
            else:
                for s in range(REG_W[r] // P):
                    total[r] += np.trace(reg[:, s * P:(s + 1) * P])
    loss_xy = total[REG_XY]
    loss_wh = total[REG_WH]
    loss_obj = total[REG_OBJ]
    loss_noobj = total[REG_PCF] - total[REG_TCPC]
    loss_cls = total[REG_CLS] + total[REG_CLT]
    losses = (np.array([loss_xy, loss_wh, loss_obj, loss_noobj, loss_cls])
              / float(N_FULL)).astype(np.float32)
    return losses


# revision 41
# speedup vs baseline: 1.2034x; 1.2034x over previous
"""YOLO-style loss kernel for Trainium2, SPMD over 8 NeuronCores.

Inputs (full): pred_tensor [32768,7,7,30] f32, target_tensor [32768,7,7,30] f32.
Output: np.ndarray shape (5,) f32 = (loss_xy, loss_wh, loss_obj, loss_noobj, loss_class).

Pure data parallel on batch: each core gets 4096 samples (200704 cells),
fp16 on-chip, box-major contiguous channel groups, 4 chunks of 392
cells/partition, raw + temp pools double-buffered.

All five sum-of-squares reductions run on the otherwise-idle TensorEngine
as gram-diagonal accumulations: PSUM[j,f] += sum_p a[p,f]*b[p,j], with the
payload on (section-)diagonals; the host extracts diagonals in fp64.
 - loss_class needs no mask op at all: gram(obj_stationary, d^2_moving)
   applies the 0/1 mask inside the PE; d^2 comes from an in-place ACT
   Square; full 128-col blocks pack 4 channel rows per matmul (out
   [128,512] = one PSUM bank), the twenty 8-wide row tails are one packed
   matmul into a dedicated [128,160] region.
 - loss_xy likewise: gram(resp_row, dxy^2); single-row grams land on the
   true diagonal.
 - loss_wh/obj keep a cheap DVE premask (resp in {0,1}) + flat self-grams.
 - loss_noobj = sum(pc^2) - sum(tc*pc^2) (tc in {0,1}): two grams, one ACT
   square, zero DVE ops.
Engine placement is latency-aware: GpSimd only gets off-critical-path
input-only products (areas) - its TT throughput is ~5x below DVE and
GP<->DVE SBUF port contention taxes the DVE, so anything on the IoU chain
stays on DVE/ACT. The corner scale rides a 4x DVE tensor_scalar. Chunk 0's
class DMA is split so the first sub starts as soon as half the data lands.
"""

import os
import sys

sys.path.insert(0, "/opt/trn_rl_repo")

import numpy as np

import concourse.bass as bass
import concourse.bacc as bacc
import concourse.tile as tile
from concourse import mybir
from concourse import bass_utils

F32 = mybir.dt.float32
F16 = mybir.dt.float16
I16 = mybir.dt.int16
I32 = mybir.dt.int32
ALU = mybir.AluOpType
ACT = mybir.ActivationFunctionType

S = 7
B = 2
C = 20
D = 30
N_FULL = 32768
N_CORES = 8
N_SHARD = N_FULL // N_CORES            # 4096 samples per core
R = N_SHARD * S * S                    # 200704 cells per core
P = 128                                # partitions
RP = R // P                            # 1568 cells per partition
NCK = 392                              # cells per partition per chunk
N_CH = RP // NCK                       # 4 chunks

PERM_XY = [0, 1, 5, 6]   # x0,y0,x1,y1
PERM_WH = [2, 3, 7, 8]   # w0,h0,w1,h1
PERM_CF = [4, 9]         # c0,c1

# gram regions in PSUM: five [128,128] + class [128,512] + class-tail [128,160]
REG_XY, REG_WH, REG_OBJ, REG_PCF, REG_TCPC, REG_CLS, REG_CLT = range(7)
REG_W = [P, P, P, P, P, 4 * P, 160]      # free width per region
N_REG = 7
OUT_W = sum(REG_W)                       # 1184 fp32 columns


def _mk(ap, dims):
    """Rebuild the free dims of `ap` (keeping partition dim + offset) as
    `dims` = list of (step, count)."""
    new = [list(ap.ap[0])] + [[s, c] for s, c in dims]
    return bass.AP(tensor=ap.tensor, offset=ap.offset, ap=new)


def _ins(ap, pos, step, count):
    new = [list(x) for x in ap.ap]
    new.insert(pos, [step, count])
    return bass.AP(tensor=ap.tensor, offset=ap.offset, ap=new)


def build_program():
    nc = bacc.Bacc("TRN2", target_bir_lowering=False, debug=False)
    n = NCK

    def din(name, per_chunk):
        return nc.dram_tensor(name, [P, N_CH * per_chunk], F16, kind="ExternalInput")

    pbox, tbox = din("pbox", n * 10), din("tbox", n * 10)
    pcl, tcl = din("pcl", C * n), din("tcl", C * n)
    out = nc.dram_tensor("out", [P, OUT_W], F32, kind="ExternalOutput")

    pbox_v = pbox.ap().rearrange("p (k a) -> p k a", k=N_CH, a=n * 10)
    tbox_v = tbox.ap().rearrange("p (k a) -> p k a", k=N_CH, a=n * 10)
    pcl_v = pcl.ap().rearrange("p (k c i) -> p k c i", k=N_CH, c=C, i=n)
    tcl_v = tcl.ap().rearrange("p (k c i) -> p k c i", k=N_CH, c=C, i=n)

    # block boundaries within one n-row (392 = 128+128+128+8)
    row_blocks = []
    o = 0
    while o < n:
        w = min(128, n - o)
        row_blocks.append((o, w))
        o += w

    with tile.TileContext(nc) as tc:
        with (
            tc.tile_pool(name="raw", bufs=2) as raw,
            tc.tile_pool(name="tmp", bufs=2) as tmp,
            tc.tile_pool(name="tmg", bufs=2) as tmg,
            tc.tile_pool(name="persist", bufs=1) as persist,
            tc.tile_pool(name="ps", bufs=1, space="PSUM") as psum,
        ):
            # gram accumulation regions
            G = [psum.tile([P, REG_W[r]], F32, name=f"G{r}", tag=f"G{r}")
                 for r in range(N_REG)]
            started = [False] * N_REG

            def gram(reg, lhsT, rhs, last):
                """PSUM[reg] += lhsT.T @ rhs (diag carries the payload)."""
                m = lhsT.free_size()
                f = rhs.free_size()
                nc.tensor.matmul(
                    G[reg][0:m, 0:f], lhsT, rhs,
                    start=not started[reg], stop=last,
                )
                started[reg] = True

            # prime the ACT table (Copy/Relu/Square/Sqrt all live in the
            # sqrt-anchored set) before any real work
            warm = persist.tile([P, 1], F16)
            nc.gpsimd.memset(warm, 1.0)
            nc.scalar.activation(warm, warm, ACT.Sqrt)

            for k in range(N_CH):
                last = k == N_CH - 1
                Bp = raw.tile([P, 10 * n], F16, tag="Bp")
                Bt = raw.tile([P, 10 * n], F16, tag="Bt")
                Pcl = raw.tile([P, C, n], F16, tag="Pcl")
                Tcl = raw.tile([P, C, n], F16, tag="Tcl")
                nc.sync.dma_start(out=Bp, in_=pbox_v[:, k])
                nc.sync.dma_start(out=Bt, in_=tbox_v[:, k])
                if k == 0:
                    for q in range(2):
                        cq = slice(10 * q, 10 * (q + 1))
                        nc.sync.dma_start(out=Pcl[:, cq, :], in_=pcl_v[:, k, cq])
                        nc.sync.dma_start(out=Tcl[:, cq, :], in_=tcl_v[:, k, cq])
                else:
                    nc.sync.dma_start(out=Pcl, in_=pcl_v[:, k])
                    nc.sync.dma_start(out=Tcl, in_=tcl_v[:, k])

                # contiguous channel-group rows
                Pxy = Bp[:, 0:4 * n]
                Pwh = Bp[:, 4 * n:8 * n]
                Pcf = Bp[:, 8 * n:10 * n]
                Txy = Bt[:, 0:4 * n]
                Twh = Bt[:, 4 * n:8 * n]
                obj_src = Bt[:, 8 * n:9 * n]    # target c0 row, [P,n]

                # ---- input-only ops first: fill every engine early ----
                t1 = tmp.tile([P, 4, n], F16, tag="t1")
                nc.vector.tensor_scalar_mul(t1, Pwh, 3.5)
                t2 = tmp.tile([P, 2, n], F16, tag="t2")
                nc.scalar.activation(t2, Bt[:, 4 * n:6 * n], ACT.Copy, scale=3.5)
                sq8 = tmp.tile([P, 8, n], F16, tag="sq8")
                nc.scalar.activation(sq8[:, 0:4, :], Pwh, ACT.Sqrt)
                nc.scalar.activation(sq8[:, 4:8, :], Twh, ACT.Sqrt)
                sqp2 = tmg.tile([P, 2, n], F16, tag="sqp2")
                nc.scalar.activation(sqp2, Pcf, ACT.Square)
                # areas on GpSimd (off the critical chain)
                areap2 = tmp.tile([P, 2, n], F16, tag="areap2")
                areat = tmp.tile([P, n], F16, tag="areat")
                pw2 = _mk(Bp[:, 4 * n], [(2 * n, 2), (1, n)])
                ph2 = _mk(Bp[:, 5 * n], [(2 * n, 2), (1, n)])
                nc.gpsimd.tensor_tensor(areap2, pw2, ph2, op=ALU.mult)
                nc.gpsimd.tensor_tensor(
                    areat, Bt[:, 4 * n:5 * n], Bt[:, 5 * n:6 * n], op=ALU.mult
                )
                def noobj_grams():
                    nb2l = (2 * n + 127) // 128
                    for bi in range(nb2l):
                        o = bi * 128
                        w = min(128, 2 * n - o)
                        gram(REG_PCF, Pcf[:, o:o + w], Pcf[:, o:o + w],
                             last and bi == nb2l - 1)
                    for r2 in range(2):
                        for bi, (o, w) in enumerate(row_blocks):
                            tcrow = Bt[:, (8 + r2) * n + o:(8 + r2) * n + o + w]
                            gram(REG_TCPC, tcrow, sqp2[:, r2, o:o + w],
                                 last and r2 == 1 and bi == len(row_blocks) - 1)

                d8 = tmg.tile([P, 8, n], F16, tag="d8")
                nc.vector.tensor_tensor(d8[:, 0:4, :], Txy, Pxy, op=ALU.subtract)
                nc.vector.tensor_tensor(
                    d8[:, 4:8, :], sq8[:, 4:8, :], sq8[:, 0:4, :], op=ALU.subtract
                )

                # ---- IoU chain (DVE-resident) ----
                nl4 = tmp.tile([P, 4, n], F16, tag="nl4")    # -(7l) both boxes
                r4 = tmp.tile([P, 4, n], F16, tag="r4")      # 7r both boxes
                nc.vector.tensor_tensor(nl4, t1, Pxy, op=ALU.subtract)
                nc.vector.tensor_tensor(r4, t1, Pxy, op=ALU.add)
                txy0 = Bt[:, 0:2 * n]
                nlt2 = tmp.tile([P, 2, n], F16, tag="nlt2")
                rt2 = tmp.tile([P, 2, n], F16, tag="rt2")
                nc.vector.tensor_tensor(nlt2, t2, txy0, op=ALU.subtract)
                nc.vector.tensor_tensor(rt2, t2, txy0, op=ALU.add)
                nlt2b = _mk(nlt2[:, 0, 0], [(0, 2), (n, 2), (1, n)])
                rt2b = _mk(rt2[:, 0, 0], [(0, 2), (n, 2), (1, n)])

                mln4 = tmp.tile([P, 4, n], F16, tag="mln4")
                mr4 = tmp.tile([P, 4, n], F16, tag="mr4")
                nc.vector.tensor_tensor(mln4, nl4, nlt2b, op=ALU.min)
                nc.vector.tensor_tensor(mr4, r4, rt2b, op=ALU.min)
                s4 = tmp.tile([P, 4, n], F16, tag="s4")
                nc.vector.tensor_tensor(s4, mln4, mr4, op=ALU.add)
                cw4 = tmp.tile([P, 4, n], F16, tag="cw4")
                nc.scalar.activation(cw4, s4, ACT.Relu, scale=1.0 / 7.0)

                # ---- class block (mid-chunk): two half-pipelines so the
                #      DVE sub of half 2 overlaps the ACT square of half 1;
                #      d^2 lands in place over Tcl ----
                H = 5 if k == 0 else C // 2
                for h in range(C // H):
                    cs = h * H
                    nc.vector.tensor_tensor(
                        Tcl[:, cs:cs + H, :], Tcl[:, cs:cs + H, :],
                        Pcl[:, cs:cs + H, :], op=ALU.subtract
                    )
                    nc.scalar.activation(
                        Tcl[:, cs:cs + H, :], Tcl[:, cs:cs + H, :], ACT.Square
                    )
                for o, w in row_blocks[:-1]:
                    for c0 in range(0, C, 4):
                        mv = _mk(Tcl[:, c0, o], [(n, 4), (1, w)])
                        gram(REG_CLS, obj_src[:, o:o + w], mv,
                             last and o == row_blocks[-2][0] and c0 == C - 4)
                to, tw = row_blocks[-1]
                mvt = _mk(Tcl[:, 0, to], [(n, C), (1, tw)])
                gram(REG_CLT, obj_src[:, to:to + tw], mvt, last)

                # ---- IoU tail ----
                inter2 = tmp.tile([P, 2, n], F16, tag="inter2")
                cwx = cw4[:, 0:4:2, :]
                cwy = cw4[:, 1:4:2, :]
                nc.vector.tensor_tensor(inter2, cwx, cwy, op=ALU.mult)
                u2h = tmp.tile([P, 2, n], F16, tag="u2h")
                u2 = tmp.tile([P, 2, n], F16, tag="u2")
                nc.vector.tensor_tensor(u2h, areap2, inter2, op=ALU.subtract)
                areatb = _ins(areat[:, :], 1, 0, 2)
                nc.vector.tensor_tensor(u2, u2h, areatb, op=ALU.add)

                from concourse.dve_ops import (
                    RECIP_APPROX_FAST_CONSTS as _RC,
                    RECIPROCAL_APPROX_FAST as _RF,
                )
                rcp16 = tmp.tile([P, 2, n], F16, tag="rcp16")
                nc.vector._custom_dve(
                    _RF, out=rcp16, in0=u2,
                    s0=_RC["s0"], s1=_RC["s1"], imm2=_RC["imm2"],
                )
                iou2 = tmp.tile([P, 2, n], F16, tag="iou2")
                nc.vector.tensor_tensor(iou2, inter2, rcp16, op=ALU.mult)

                is1 = tmp.tile([P, n], F16, tag="is1")
                riou = tmp.tile([P, n], F16, tag="riou")
                nc.vector.tensor_tensor(is1, iou2[:, 1, :], iou2[:, 0, :], op=ALU.is_gt)
                nc.vector.tensor_tensor(riou, iou2[:, 1, :], iou2[:, 0, :], op=ALU.max)

                resp = tmp.tile([P, 2, n], F16, tag="resp")
                nc.vector.tensor_tensor(resp[:, 1, :], obj_src, is1, op=ALU.mult)
                nc.vector.tensor_tensor(resp[:, 0, :], obj_src, resp[:, 1, :], op=ALU.subtract)

                # ---- xy: square d8 rows 0:4 in place (ACT), resp applied
                #      by the PE (single-row grams land on the true diag) ----
                nc.scalar.activation(d8[:, 0:4, :], d8[:, 0:4, :], ACT.Square)
                for r in range(4):
                    bx = r // 2
                    for bi, (o, w) in enumerate(row_blocks):
                        gram(REG_XY, resp[:, bx, o:o + w], d8[:, r, o:o + w],
                             last and r == 3 and bi == len(row_blocks) - 1)
                # ---- wh: premask in place + flat self-gram ----
                resp4b = _mk(resp[:, 0, 0], [(n, 2), (0, 2), (1, n)])
                nc.vector.tensor_tensor(d8[:, 4:8, :], d8[:, 4:8, :], resp4b, op=ALU.mult)
                whf = _mk(d8[:, 4, 0], [(1, 4 * n)])
                nb = (4 * n + 127) // 128
                for bi in range(nb):
                    o = bi * 128
                    w = min(128, 4 * n - o)
                    gram(REG_WH, whf[:, o:o + w], whf[:, o:o + w],
                         last and bi == nb - 1)

                # ---- obj conf: dc2 = riou - pc (GP), premask, self-gram ----
                dc2 = tmg.tile([P, 2, n], F16, tag="dc2")
                rioub = _ins(riou[:, :], 1, 0, 2)
                nc.vector.tensor_tensor(dc2, rioub, Pcf, op=ALU.subtract)
                nc.vector.tensor_tensor(dc2, dc2, resp, op=ALU.mult)
                dmc2f = _mk(dc2[:, 0, 0], [(1, 2 * n)])
                nb2 = (2 * n + 127) // 128
                for bi in range(nb2):
                    o = bi * 128
                    w = min(128, 2 * n - o)
                    gram(REG_OBJ, dmc2f[:, o:o + w], dmc2f[:, o:o + w],
                         last and bi == nb2 - 1)

                # ---- noobj: sum(pc^2) - sum(tc*pc^2), tc in {0,1} ----
                noobj_grams()

            # ---- extract: copy PSUM regions to SBUF, DMA each out as
            #      soon as its copy lands (only REG_OBJ gates the tail) ----
            og = persist.tile([P, OUT_W], F32)
            outv = out.ap()
            off = 0
            for r in range(N_REG):
                nc.scalar.activation(og[:, off:off + REG_W[r]], G[r], ACT.Copy)
                nc.sync.dma_start(out=outv[:, off:off + REG_W[r]],
                                  in_=og[:, off:off + REG_W[r]])
                off += REG_W[r]

    nc.compile()
    return nc


_nc_cache = None
LAST_EXEC_NS = None
LAST_RESULT = None


def _get_nc():
    global _nc_cache
    if _nc_cache is None:
        _nc_cache = build_program()
    return _nc_cache


def _prep(full):
    """[N*S*S, 30] f32 -> per-core fp16 (box blocks [k][xy4|wh4|cf2], cls)."""
    A = np.asarray(full, dtype=np.float32).reshape(N_CORES, P, N_CH, NCK, D)
    A16 = A.astype(np.float16)
    xy = A16[..., PERM_XY].transpose(0, 1, 2, 4, 3)
    wh = A16[..., PERM_WH].transpose(0, 1, 2, 4, 3)
    cf = A16[..., PERM_CF].transpose(0, 1, 2, 4, 3)
    box = np.ascontiguousarray(np.concatenate([xy, wh, cf], axis=-2)).reshape(
        N_CORES, P, -1
    )
    cl = np.ascontiguousarray(A16[..., 10:30].transpose(0, 1, 2, 4, 3)).reshape(
        N_CORES, P, -1
    )
    return box, cl


def kernel(pred_tensor, target_tensor):
    global LAST_EXEC_NS, LAST_RESULT
    pred = np.asarray(pred_tensor).reshape(N_FULL * S * S, D)
    tgt = np.asarray(target_tensor).reshape(N_FULL * S * S, D)

    pb, pc = _prep(pred)
    tb, tc = _prep(tgt)

    in_maps = []
    for i in range(N_CORES):
        in_maps.append({"pbox": pb[i], "tbox": tb[i], "pcl": pc[i], "tcl": tc[i]})

    nc = _get_nc()
    trace = bool(os.environ.get("KERNEL_TRACE"))
    tmpdir = os.environ.get("KERNEL_TRACE_DIR") or None
    res = bass_utils.run_bass_kernel_spmd(
        nc, in_maps, core_ids=list(range(N_CORES)), trace=trace, tmpdir=tmpdir
    )
    LAST_RESULT = res
    if res.exec_time_ns is not None:
        LAST_EXEC_NS = res.exec_time_ns
    # gram regions; (section-)diagonals carry the sums
    offs = np.cumsum([0] + REG_W)
    total = np.zeros(N_REG, dtype=np.float64)
    for m in res.results:
        o = m["out"].astype(np.float64)
        for r in range(N_REG):
            reg = o[:, offs[r]:offs[r + 1]]
            if r == REG_CLT:
                for c in range(C):
                    total[r] += np.trace(reg[:8, c * 8:(c + 1) * 8])
            else:
                for s in range(REG_W[r] // P):
                    total[r] += np.trace(reg[:, s * P:(s + 1) * P])
    loss_xy = total[REG_XY]
    loss_wh = total[REG_WH]
    loss_obj = total[REG_OBJ]
    loss_noobj = total[REG_PCF] - total[REG_TCPC]
    loss_cls = total[REG_CLS] + total[REG_CLT]
    losses = (np.array([loss_xy, loss_wh, loss_obj, loss_noobj, loss_cls])
              / float(N_FULL)).astype(np.float32)
    return losses
